# revision 7
# baseline (speedup 1.0000x reference)
"""Trainium2 Bass kernel for nn_CustomNet_30966714204481.

Model: LSTM(40->100, T=4096, batch=16, keep last h) -> Linear(100,100)
       -> BatchNorm1d(train stats over batch) -> Linear(100,40) -> reshape.

Strategy:
  * Data-parallel: batch 16 split as 2 sequences per NeuronCore x 8 cores.
  * Gates-on-partitions layout: all per-step tensors are [100 part, B] so
    ACT/DVE fixed costs amortize over 100 lanes.
  * Input projections xg = W_ih @ x (+biases, via an appended ones-row on x)
    are computed by the tensor engine directly into PSUM in windows of 64
    timesteps (one bank), strided so each step's 4 gates x B columns are
    contiguous. The per-step recurrent matmuls accumulate on top
    (has_written bits), so no separate add is on the serial critical path.
  * Gate order permuted to (i, f, o, g) and the g-gate rows pre-scaled by 2
    host-side so ONE sigmoid per step covers all gates:
    tanh(z) = 2*sigmoid(2z) - 1, fixed up inside fused DVE ops.
  * Per-step serial chain: 4 matmuls -> sigmoid(ACT) -> 3 fused DVE ops for
    the cell update -> tanh(ACT) -> 1 DVE mul for h.
  * BatchNorm tail: per-core local sums + tiny AllReduce, tail linears on
    device, each core outputs its own [40, B] slice (gathered on host).
"""

import numpy as np
from contextlib import ExitStack

H = 100
F = 40
FA = F + 1  # +1 ones-row that carries the biases through the x-projection
G4 = 4 * H
B_TOT = 16
N_CORES = 8
B = B_TOT // N_CORES  # 2 sequences per core
T = 4096
EPS = 1e-5
WS = 64  # timesteps per PSUM window (WS * 4 * B = 512 fp32 = one bank)


def build_module(t_local=T, b_local=B, device_tail=True, n_cores=N_CORES):
    import concourse.bacc as bacc
    import concourse.tile as tile
    import concourse.mybir as mybir

    f32 = mybir.dt.float32
    AF = mybir.ActivationFunctionType
    OP = mybir.AluOpType

    sc = 4 * b_local  # z columns per step
    ws = min(WS, t_local)
    assert t_local % ws == 0
    n_win = t_local // ws
    assert ws * sc <= 512  # one PSUM bank

    nc = bacc.Bacc("TRN2", target_bir_lowering=False, debug=False,
                   num_devices=n_cores)

    x_d = nc.declare_dram_parameter("x", [FA, t_local * b_local], f32, isOutput=False)
    wih_d = nc.declare_dram_parameter("wih", [FA, G4], f32, isOutput=False)
    whh_d = nc.declare_dram_parameter("whh", [H, G4], f32, isOutput=False)
    w1_d = nc.declare_dram_parameter("w1", [H, H], f32, isOutput=False)
    b1_d = nc.declare_dram_parameter("b1", [H, 1], f32, isOutput=False)
    gam_d = nc.declare_dram_parameter("gamma", [H, 1], f32, isOutput=False)
    bet_d = nc.declare_dram_parameter("beta", [H, 1], f32, isOutput=False)
    w2_d = nc.declare_dram_parameter("w2", [H, F], f32, isOutput=False)
    b2_d = nc.declare_dram_parameter("b2", [F, 1], f32, isOutput=False)
    h_d = nc.declare_dram_parameter("hout", [H, b_local], f32, isOutput=True)
    out_d = nc.declare_dram_parameter("out", [F, b_local], f32, isOutput=True)

    with tile.TileContext(nc, num_cores=n_cores) as tc, ExitStack() as ctx:
        consts = ctx.enter_context(tc.tile_pool(name="consts", bufs=1))
        state = ctx.enter_context(tc.tile_pool(name="state", bufs=1))
        upool = ctx.enter_context(tc.tile_pool(name="upool", bufs=4))
        tmp = ctx.enter_context(tc.tile_pool(name="tmp", bufs=4))
        zpool = ctx.enter_context(
            tc.tile_pool(name="zpool", bufs=2, space="PSUM"))

        x_sb = consts.tile([FA, t_local * b_local], f32)
        nc.sync.dma_start(x_sb[:], x_d[:])
        wih_sb = consts.tile([FA, G4], f32)
        nc.sync.dma_start(wih_sb[:], wih_d[:])
        whh_sb = consts.tile([H, G4], f32)
        nc.sync.dma_start(whh_sb[:], whh_d[:])

        h = state.tile([H, b_local], f32)
        nc.vector.memset(h[:], 0.0)
        c = state.tile([H, b_local], f32)
        nc.vector.memset(c[:], 0.0)

        # Gate order after host permutation: (f, i, o, g).
        # The f-gate z goes to its own PSUM bank so sigmoid(f) can read it
        # while the PE is still writing the other gates' bank; r = f*c then
        # hides under the PE segment.
        # All nonlinearities are Sigmoid (tanh(x) = 2*sig(2x)-1): the device
        # h-state is h/2 = (sig(2c)-0.5)*o, compensated host-side by
        # doubling W_hh and W1.
        for w in range(n_win):
            zf = zpool.tile([H, ws, b_local], f32, tag="zf")
            zx = zpool.tile([H, 3, ws, b_local], f32, tag="zx")
            rhs_x = x_sb[:, w * ws * b_local:(w + 1) * ws * b_local]
            nc.tensor.matmul(
                zf[:, :, :], lhsT=wih_sb[:, 0:H], rhs=rhs_x,
                start=True, stop=False, skip_group_check=True)
            for g in range(3):
                nc.tensor.matmul(
                    zx[:, g, :, :],
                    lhsT=wih_sb[:, (g + 1) * H:(g + 2) * H],
                    rhs=rhs_x,
                    start=(g == 0), stop=False, skip_group_check=True)
            for s in range(ws):
                nc.tensor.matmul(
                    zf[:, s, :], lhsT=whh_sb[:, 0:H], rhs=h[:],
                    start=False, stop=True, skip_group_check=True)
                for g in range(3):
                    nc.tensor.matmul(
                        zx[:, g, s, :],
                        lhsT=whh_sb[:, (g + 1) * H:(g + 2) * H],
                        rhs=h[:],
                        start=False, stop=(g == 2), skip_group_check=True)
                uf = upool.tile([H, b_local], f32, tag="uf")
                nc.scalar.activation(uf[:], zf[:, s, :], AF.Sigmoid)
                r = tmp.tile([H, b_local], f32, tag="r")
                nc.vector.tensor_mul(r[:], uf[:], c[:])
                u3 = upool.tile([H, 3, b_local], f32, tag="u3")
                nc.scalar.activation(u3[:], zx[:, :, s, :], AF.Sigmoid)
                ui = u3[:, 0, :]
                uo = u3[:, 1, :]
                ug = u3[:, 2, :]
                # qp = (sig(2 z_g) - 0.5) * i == tanh(z_g) * i / 2
                qp = tmp.tile([H, b_local], f32, tag="qp")
                nc.vector.scalar_tensor_tensor(
                    qp[:], ug, 0.5, ui, OP.subtract, OP.mult)
                # c = 2*qp + r = i*tanh(z_g) + f*c
                nc.vector.scalar_tensor_tensor(
                    c[:], qp[:], 2.0, r[:], OP.mult, OP.add)
                # uc = sig(2c); h_dev = (uc-0.5)*o = o*tanh(c)/2
                uc = tmp.tile([H, b_local], f32, tag="uc")
                nc.scalar.activation(uc[:], c[:], AF.Sigmoid, scale=2.0)
                nc.vector.scalar_tensor_tensor(
                    h[:], uc[:], 0.5, uo, OP.subtract, OP.mult)

        nc.sync.dma_start(h_d[:], h[:])

        if device_tail:
            w1_sb = consts.tile([H, H], f32)
            nc.sync.dma_start(w1_sb[:], w1_d[:])
            b1_sb = consts.tile([H, 1], f32)
            nc.sync.dma_start(b1_sb[:], b1_d[:])
            gam_sb = consts.tile([H, 1], f32)
            nc.sync.dma_start(gam_sb[:], gam_d[:])
            bet_sb = consts.tile([H, 1], f32)
            nc.sync.dma_start(bet_sb[:], bet_d[:])
            w2_sb = consts.tile([H, F], f32)
            nc.sync.dma_start(w2_sb[:], w2_d[:])
            b2_sb = consts.tile([F, 1], f32)
            nc.sync.dma_start(b2_sb[:], b2_d[:])

            y1p = zpool.tile([H, b_local], f32)
            nc.tensor.matmul(y1p[:], lhsT=w1_sb[:], rhs=h[:],
                             start=True, stop=True)
            y1 = tmp.tile([H, b_local], f32, tag="y1")
            nc.scalar.activation(y1[:], y1p[:], AF.Identity, bias=b1_sb[:])

            # local batch stats: sum(y) and sum(y^2) over the B local cols
            st = tmp.tile([H, 2], f32, tag="st")
            nc.vector.reduce_sum(st[:, 0:1], y1[:], axis=mybir.AxisListType.X)
            ysq = tmp.tile([H, b_local], f32, tag="ysq")
            nc.vector.tensor_mul(ysq[:], y1[:], y1[:])
            nc.vector.reduce_sum(st[:, 1:2], ysq[:], axis=mybir.AxisListType.X)

            if n_cores > 1:
                dpool = ctx.enter_context(
                    tc.tile_pool(name="dram", bufs=1, space="DRAM"))
                st_in = dpool.tile([H, 2], f32)
                st_out = dpool.tile([H, 2], f32)
                nc.sync.dma_start(st_in[:], st[:])
                nc.gpsimd.collective_compute(
                    "AllReduce", OP.add,
                    replica_groups=[list(range(n_cores))],
                    ins=[st_in.opt()], outs=[st_out.opt()])
                stg = tmp.tile([H, 2], f32, tag="stg")
                nc.sync.dma_start(stg[:], st_out[:])
            else:
                stg = st

            mom = tmp.tile([H, 2], f32, tag="mom")
            nc.vector.tensor_scalar_mul(mom[:], stg[:], 1.0 / B_TOT)
            mu = mom[:, 0:1]
            musq = tmp.tile([H, 1], f32, tag="musq")
            nc.vector.tensor_mul(musq[:], mu, mu)
            var = tmp.tile([H, 1], f32, tag="var")
            # var = E[y^2] - mu^2 + eps
            nc.vector.scalar_tensor_tensor(
                var[:], mom[:, 1:2], EPS, musq[:], OP.add, OP.subtract)
            sd = tmp.tile([H, 1], f32, tag="sd")
            nc.scalar.sqrt(sd[:], var[:])
            rstd = tmp.tile([H, 1], f32, tag="rstd")
            nc.vector.reciprocal(rstd[:], sd[:])

            yh = tmp.tile([H, b_local], f32, tag="yh")
            nc.vector.tensor_scalar(
                yh[:], y1[:], mu, rstd[:], OP.subtract, OP.mult)
            yn = tmp.tile([H, b_local], f32, tag="yn")
            nc.vector.tensor_scalar(
                yn[:], yh[:], gam_sb[:], bet_sb[:], OP.mult, OP.add)

            y2p = zpool.tile([F, b_local], f32)
            nc.tensor.matmul(y2p[:], lhsT=w2_sb[:], rhs=yn[:],
                             start=True, stop=True)
            y2 = tmp.tile([F, b_local], f32, tag="y2")
            nc.scalar.activation(y2[:], y2p[:], AF.Identity, bias=b2_sb[:])
            nc.sync.dma_start(out_d[:], y2[:])
        else:
            zero = tmp.tile([F, b_local], f32, tag="zero")
            nc.vector.memset(zero[:], 0.0)
            nc.sync.dma_start(out_d[:], zero[:])

    nc.compile()
    return nc


def prep_weights(W_ih, W_hh, b_ih, b_hh):
    """Permute gates (i,f,g,o)->(f,i,o,g), scale g rows by 2 (sigmoid trick),
    double W_hh (device h-state is h/2), fold biases into an extra x row."""
    perm = np.concatenate(
        [np.arange(100, 200), np.arange(0, 100),
         np.arange(300, 400), np.arange(200, 300)])
    scale = np.ones((G4, 1), np.float32)
    scale[300:400] = 2.0  # g block sits last after the permutation
    wih_p = W_ih[perm] * scale          # [400, 40]
    whh_p = W_hh[perm] * scale * 2.0    # [400, 100]
    bias_p = (b_ih + b_hh)[perm] * scale[:, 0]  # [400]
    wih_aug = np.concatenate(
        [wih_p.T, bias_p[None, :]], axis=0).astype(np.float32)  # [41, 400]
    whh_t = np.ascontiguousarray(whh_p.T).astype(np.float32)    # [100, 400]
    return wih_aug, whh_t


def prep_x_core(x_core):
    """[B, T, F] -> [FA, T*B] with column order t*B+b and a ones-row."""
    b_local, t_local, _ = x_core.shape
    xt = np.ascontiguousarray(
        x_core.transpose(2, 1, 0).reshape(F, t_local * b_local))
    return np.concatenate(
        [xt, np.ones((1, t_local * b_local), np.float32)], axis=0)


_MODULE_CACHE = {}


def get_module(**kw):
    key = tuple(sorted(kw.items()))
    if key not in _MODULE_CACHE:
        _MODULE_CACHE[key] = build_module(**kw)
    return _MODULE_CACHE[key]


def make_in_maps(inputs, n_cores=N_CORES):
    wih_aug, whh_t = prep_weights(
        inputs["W_ih"], inputs["W_hh"], inputs["b_ih"], inputs["b_hh"])
    com = {
        "wih": wih_aug,
        "whh": whh_t,
        "w1": np.ascontiguousarray(2.0 * inputs["W1"].T).astype(np.float32),
        "b1": inputs["b1"].reshape(H, 1).astype(np.float32),
        "gamma": inputs["gamma"].reshape(H, 1).astype(np.float32),
        "beta": inputs["beta"].reshape(H, 1).astype(np.float32),
        "w2": np.ascontiguousarray(inputs["W2"].T).astype(np.float32),
        "b2": inputs["b2"].reshape(F, 1).astype(np.float32),
    }
    x = np.asarray(inputs["x"], np.float32)
    b_per = x.shape[0] // n_cores
    return [
        {**com, "x": prep_x_core(x[i * b_per:(i + 1) * b_per])}
        for i in range(n_cores)
    ]


def kernel(**inputs):
    from concourse.bass_utils import run_bass_kernel_spmd

    nc = get_module()
    in_maps = make_in_maps(inputs)
    res = run_bass_kernel_spmd(nc, in_maps, list(range(N_CORES)))
    y = np.concatenate(
        [res.results[i]["out"].T for i in range(N_CORES)], axis=0)  # [16, 40]
    return np.ascontiguousarray(y.reshape(B_TOT, 10, 4).astype(np.float32))


# revision 18
# speedup vs baseline: 1.6317x; 1.6317x over previous
"""Trainium2 Bass kernel for nn_CustomNet_30966714204481.

Model: LSTM(40->100, T=4096, batch=16, keep last h) -> Linear(100,100)
       -> BatchNorm1d(train stats over batch) -> Linear(100,40) -> reshape.

Strategy:
  * Data-parallel: batch 16 split as 2 sequences per NeuronCore x 8 cores.
  * Gates-on-partitions layout: all per-step tensors are [100 part, B] so
    ACT/DVE fixed costs amortize over 100 lanes.
  * Input projections xg = W_ih @ x (+biases, via an appended ones-row on x)
    are computed by the tensor engine directly into PSUM in windows of 64
    timesteps (one bank), strided so each step's 4 gates x B columns are
    contiguous. The per-step recurrent matmuls accumulate on top
    (has_written bits), so no separate add is on the serial critical path.
  * Gate order permuted to (i, f, o, g) and the g-gate rows pre-scaled by 2
    host-side so ONE sigmoid per step covers all gates:
    tanh(z) = 2*sigmoid(2z) - 1, fixed up inside fused DVE ops.
  * Per-step serial chain: 4 matmuls -> sigmoid(ACT) -> 3 fused DVE ops for
    the cell update -> tanh(ACT) -> 1 DVE mul for h.
  * BatchNorm tail: per-core local sums + tiny AllReduce, tail linears on
    device, each core outputs its own [40, B] slice (gathered on host).
"""

import numpy as np
from contextlib import ExitStack

H = 100
F = 40
FA = F + 1  # +1 ones-row that carries the biases through the x-projection
G4 = 4 * H
B_TOT = 16
N_CORES = 8
B = B_TOT // N_CORES  # 2 sequences per core
T = 4096
EPS = 1e-5
WS = 64  # timesteps per PSUM window (WS * 4 * B = 512 fp32 = one bank)


def build_module(t_local=T, b_local=B, device_tail=True, n_cores=N_CORES):
    import concourse.bacc as bacc
    import concourse.tile as tile
    import concourse.mybir as mybir

    f32 = mybir.dt.float32
    bf16 = mybir.dt.float16  # fp16: finer mantissa than bf16, same PE speed
    AF = mybir.ActivationFunctionType
    OP = mybir.AluOpType
    MP = 128  # gate weight M padded to 128 so bf16 fast-weight-load engages

    sc = 4 * b_local  # z columns per step
    ws = min(WS, t_local)
    assert t_local % ws == 0
    n_win = t_local // ws
    assert ws * sc <= 512  # one PSUM bank

    nc = bacc.Bacc("TRN2", target_bir_lowering=False, debug=False,
                   num_devices=n_cores)

    x_d = nc.declare_dram_parameter("x", [FA, t_local * b_local], f32, isOutput=False)
    wih_d = nc.declare_dram_parameter("wih", [FA, 4, MP], f32, isOutput=False)
    whh_d = nc.declare_dram_parameter("whh", [H, 4, MP], bf16, isOutput=False)
    w1_d = nc.declare_dram_parameter("w1", [H, H], f32, isOutput=False)
    b1_d = nc.declare_dram_parameter("b1", [H, 1], f32, isOutput=False)
    gam_d = nc.declare_dram_parameter("gamma", [H, 1], f32, isOutput=False)
    bet_d = nc.declare_dram_parameter("beta", [H, 1], f32, isOutput=False)
    w2_d = nc.declare_dram_parameter("w2", [H, F], f32, isOutput=False)
    b2_d = nc.declare_dram_parameter("b2", [F, 1], f32, isOutput=False)
    h_d = nc.declare_dram_parameter("hout", [H, b_local], f32, isOutput=True)
    out_d = nc.declare_dram_parameter("out", [F, b_local], f32, isOutput=True)

    with tile.TileContext(nc, num_cores=n_cores) as tc, ExitStack() as ctx:
        consts = ctx.enter_context(tc.tile_pool(name="consts", bufs=1))
        state = ctx.enter_context(tc.tile_pool(name="state", bufs=1))
        upool = ctx.enter_context(tc.tile_pool(name="upool", bufs=4))
        tmp = ctx.enter_context(tc.tile_pool(name="tmp", bufs=4))
        zpool = ctx.enter_context(
            tc.tile_pool(name="zpool", bufs=2, space="PSUM"))

        x_sb = consts.tile([FA, t_local * b_local], f32)
        nc.sync.dma_start(x_sb[:], x_d[:])
        wih_sb = consts.tile([FA, 4, MP], f32)
        nc.sync.dma_start(wih_sb[:], wih_d[:])
        whh_sb = consts.tile([H, 4, MP], bf16)
        nc.sync.dma_start(whh_sb[:], whh_d[:])

        h = state.tile([H, b_local], bf16)
        nc.vector.memset(h[:], 0.0)
        c = state.tile([H, b_local], f32)
        nc.vector.memset(c[:], 0.0)

        # Gate order after host permutation: (f, i, o, g).
        # The f-gate z goes to its own PSUM bank so sigmoid(f) can read it
        # while the PE is still writing the other gates' bank; r = f*c then
        # hides under the PE segment.
        # All nonlinearities are Sigmoid (tanh(x) = 2*sig(2x)-1): the device
        # h-state is h/2 = (sig(2c)-0.5)*o, compensated host-side by
        # doubling W_hh and W1.
        for w in range(n_win):
            zf = zpool.tile([MP, ws, b_local], f32, tag="zf")
            zx = zpool.tile([MP, 3, ws, b_local], f32, tag="zx")
            rhs_x = x_sb[:, w * ws * b_local:(w + 1) * ws * b_local]
            nc.tensor.matmul(
                zf[:, :, :], lhsT=wih_sb[:, 0, :], rhs=rhs_x,
                start=True, stop=False, skip_group_check=True)
            for g in range(3):
                nc.tensor.matmul(
                    zx[:, g, :, :],
                    lhsT=wih_sb[:, g + 1, :],
                    rhs=rhs_x,
                    start=(g == 0), stop=False, skip_group_check=True)
            for s in range(ws):
                nc.tensor.matmul(
                    zf[:, s, :], lhsT=whh_sb[:, 0, :], rhs=h[:],
                    start=False, stop=True, skip_group_check=True)
                for g in range(3):
                    nc.tensor.matmul(
                        zx[:, g, s, :],
                        lhsT=whh_sb[:, g + 1, :],
                        rhs=h[:],
                        start=False, stop=(g == 2), skip_group_check=True)
                uf = upool.tile([H, b_local], f32, tag="uf")
                nc.scalar.activation(uf[:], zf[0:H, s, :], AF.Sigmoid)
                r = tmp.tile([H, b_local], f32, tag="r")
                nc.vector.tensor_mul(r[:], uf[:], c[:])
                u3 = upool.tile([H, 3, b_local], f32, tag="u3")
                nc.scalar.activation(u3[:], zx[0:H, :, s, :], AF.Sigmoid)
                ui = u3[:, 0, :]
                uo = u3[:, 1, :]
                ug = u3[:, 2, :]
                # qp = (sig(2 z_g) - 0.5) * i == tanh(z_g) * i / 2
                qp = tmp.tile([H, b_local], f32, tag="qp")
                nc.vector.scalar_tensor_tensor(
                    qp[:], ug, 0.5, ui, OP.subtract, OP.mult)
                # c = 2*qp + r = i*tanh(z_g) + f*c
                nc.vector.scalar_tensor_tensor(
                    c[:], qp[:], 2.0, r[:], OP.mult, OP.add)
                # uc = sig(2c); h_dev = (uc-0.5)*o = o*tanh(c)/2
                uc = tmp.tile([H, b_local], f32, tag="uc")
                nc.scalar.activation(uc[:], c[:], AF.Sigmoid, scale=2.0)
                nc.vector.scalar_tensor_tensor(
                    h[:], uc[:], 0.5, uo, OP.subtract, OP.mult)

        h32 = state.tile([H, b_local], f32)
        nc.vector.tensor_copy(h32[:], h[:])
        nc.sync.dma_start(h_d[:], h32[:])

        if device_tail:
            w1_sb = consts.tile([H, H], f32)
            nc.sync.dma_start(w1_sb[:], w1_d[:])
            b1_sb = consts.tile([H, 1], f32)
            nc.sync.dma_start(b1_sb[:], b1_d[:])
            gam_sb = consts.tile([H, 1], f32)
            nc.sync.dma_start(gam_sb[:], gam_d[:])
            bet_sb = consts.tile([H, 1], f32)
            nc.sync.dma_start(bet_sb[:], bet_d[:])
            w2_sb = consts.tile([H, F], f32)
            nc.sync.dma_start(w2_sb[:], w2_d[:])
            b2_sb = consts.tile([F, 1], f32)
            nc.sync.dma_start(b2_sb[:], b2_d[:])

            y1p = zpool.tile([H, b_local], f32)
            nc.tensor.matmul(y1p[:], lhsT=w1_sb[:], rhs=h32[:],
                             start=True, stop=True)
            y1 = tmp.tile([H, b_local], f32, tag="y1")
            nc.scalar.activation(y1[:], y1p[:], AF.Identity, bias=b1_sb[:])

            # local batch stats: sum(y) and sum(y^2) over the B local cols
            st = tmp.tile([H, 2], f32, tag="st")
            nc.vector.reduce_sum(st[:, 0:1], y1[:], axis=mybir.AxisListType.X)
            ysq = tmp.tile([H, b_local], f32, tag="ysq")
            nc.vector.tensor_mul(ysq[:], y1[:], y1[:])
            nc.vector.reduce_sum(st[:, 1:2], ysq[:], axis=mybir.AxisListType.X)

            if n_cores > 1:
                dpool = ctx.enter_context(
                    tc.tile_pool(name="dram", bufs=1, space="DRAM"))
                st_in = dpool.tile([H, 2], f32)
                st_out = dpool.tile([H, 2], f32)
                nc.sync.dma_start(st_in[:], st[:])
                nc.gpsimd.collective_compute(
                    "AllReduce", OP.add,
                    replica_groups=[list(range(n_cores))],
                    ins=[st_in.opt()], outs=[st_out.opt()])
                stg = tmp.tile([H, 2], f32, tag="stg")
                nc.sync.dma_start(stg[:], st_out[:])
            else:
                stg = st

            mom = tmp.tile([H, 2], f32, tag="mom")
            nc.vector.tensor_scalar_mul(mom[:], stg[:], 1.0 / B_TOT)
            mu = mom[:, 0:1]
            musq = tmp.tile([H, 1], f32, tag="musq")
            nc.vector.tensor_mul(musq[:], mu, mu)
            var = tmp.tile([H, 1], f32, tag="var")
            # var = E[y^2] - mu^2 + eps
            nc.vector.scalar_tensor_tensor(
                var[:], mom[:, 1:2], EPS, musq[:], OP.add, OP.subtract)
            sd = tmp.tile([H, 1], f32, tag="sd")
            nc.scalar.sqrt(sd[:], var[:])
            rstd = tmp.tile([H, 1], f32, tag="rstd")
            nc.vector.reciprocal(rstd[:], sd[:])

            yh = tmp.tile([H, b_local], f32, tag="yh")
            nc.vector.tensor_scalar(
                yh[:], y1[:], mu, rstd[:], OP.subtract, OP.mult)
            yn = tmp.tile([H, b_local], f32, tag="yn")
            nc.vector.tensor_scalar(
                yn[:], yh[:], gam_sb[:], bet_sb[:], OP.mult, OP.add)

            y2p = zpool.tile([F, b_local], f32)
            nc.tensor.matmul(y2p[:], lhsT=w2_sb[:], rhs=yn[:],
                             start=True, stop=True)
            y2 = tmp.tile([F, b_local], f32, tag="y2")
            nc.scalar.activation(y2[:], y2p[:], AF.Identity, bias=b2_sb[:])
            nc.sync.dma_start(out_d[:], y2[:])
        else:
            zero = tmp.tile([F, b_local], f32, tag="zero")
            nc.vector.memset(zero[:], 0.0)
            nc.sync.dma_start(out_d[:], zero[:])

    nc.compile()
    return nc


def prep_weights(W_ih, W_hh, b_ih, b_hh):
    """Permute gates (i,f,g,o)->(f,i,o,g), scale g rows by 2 (sigmoid trick),
    double W_hh (device h-state is h/2), fold biases into an extra x row."""
    perm = np.concatenate(
        [np.arange(100, 200), np.arange(0, 100),
         np.arange(300, 400), np.arange(200, 300)])
    scale = np.ones((G4, 1), np.float32)
    scale[300:400] = 2.0  # g block sits last after the permutation
    wih_p = W_ih[perm] * scale          # [400, 40]
    whh_p = W_hh[perm] * scale * 2.0    # [400, 100]
    bias_p = (b_ih + b_hh)[perm] * scale[:, 0]  # [400]
    wih_aug = np.zeros((FA, 4, 128), np.float32)
    wih_aug[:, :, :H] = np.concatenate(
        [wih_p.T, bias_p[None, :]], axis=0).reshape(FA, 4, H)
    # W_hh as fp16 lhsT, gate-major, M padded 100 -> 128 for fast weight load
    whh_t = np.zeros((H, 4, 128), np.float16)
    whh_t[:, :, :H] = whh_p.T.reshape(H, 4, H).astype(np.float16)
    return wih_aug, whh_t


def prep_x_core(x_core):
    """[B, T, F] -> [FA, T*B] with column order t*B+b and a ones-row."""
    b_local, t_local, _ = x_core.shape
    xt = np.ascontiguousarray(
        x_core.transpose(2, 1, 0).reshape(F, t_local * b_local))
    return np.concatenate(
        [xt, np.ones((1, t_local * b_local), np.float32)], axis=0)


_MODULE_CACHE = {}


def get_module(**kw):
    key = tuple(sorted(kw.items()))
    if key not in _MODULE_CACHE:
        _MODULE_CACHE[key] = build_module(**kw)
    return _MODULE_CACHE[key]


def make_in_maps(inputs, n_cores=N_CORES):
    wih_aug, whh_t = prep_weights(
        inputs["W_ih"], inputs["W_hh"], inputs["b_ih"], inputs["b_hh"])
    com = {
        "wih": wih_aug,
        "whh": whh_t,
        "w1": np.ascontiguousarray(2.0 * inputs["W1"].T).astype(np.float32),
        "b1": inputs["b1"].reshape(H, 1).astype(np.float32),
        "gamma": inputs["gamma"].reshape(H, 1).astype(np.float32),
        "beta": inputs["beta"].reshape(H, 1).astype(np.float32),
        "w2": np.ascontiguousarray(inputs["W2"].T).astype(np.float32),
        "b2": inputs["b2"].reshape(F, 1).astype(np.float32),
    }
    x = np.asarray(inputs["x"], np.float32)
    b_per = x.shape[0] // n_cores
    return [
        {**com, "x": prep_x_core(x[i * b_per:(i + 1) * b_per])}
        for i in range(n_cores)
    ]


def kernel(**inputs):
    from concourse.bass_utils import run_bass_kernel_spmd

    nc = get_module()
    in_maps = make_in_maps(inputs)
    res = run_bass_kernel_spmd(nc, in_maps, list(range(N_CORES)))
    y = np.concatenate(
        [res.results[i]["out"].T for i in range(N_CORES)], axis=0)  # [16, 40]
    return np.ascontiguousarray(y.reshape(B_TOT, 10, 4).astype(np.float32))
